# revision 2
# baseline (speedup 1.0000x reference)
"""Bistride graph message passing on 8 Trainium2 NeuronCores.

Strategy: all dense MLP compute (matmuls + bias + SiLU — ~99% of FLOPs) runs
on device via one fused 3-layer-MLP Bass/Tile kernel, row-sharded across the
8 cores (edge-parallel for edge MLPs, node-parallel for node MLPs, per the
sharding hint). Host numpy does index preprocessing, gathers/concats,
segment-sum scatters, LayerNorm and pooling glue.
"""

import numpy as np

import concourse.bacc as bacc
import concourse.tile as tile
from concourse import mybir
from concourse.bass_utils import run_bass_kernel_spmd

LAT = 128
DEPTH = 2
N_CORES = 8
KMAX = 260  # edge MLP input dim (2*128+3+1); node MLP (256) is zero-padded
MC = 40960  # rows per core per launch (max real: 320000/8 = 40000)
TILE_N = 512
F32 = mybir.dt.float32

LAST_HW_NS = 0  # accumulated wall time of device launches (informational)


def _build_kernel():
    nc = bacc.Bacc(
        "TRN2", target_bir_lowering=False, debug=False, num_devices=N_CORES
    )
    xT = nc.dram_tensor("xT", [KMAX, MC], F32, kind="ExternalInput")
    wa = nc.dram_tensor("wa", [128, 128], F32, kind="ExternalInput")
    wb = nc.dram_tensor("wb", [128, 128], F32, kind="ExternalInput")
    wc = nc.dram_tensor("wc", [KMAX - 256, 128], F32, kind="ExternalInput")
    w1 = nc.dram_tensor("w1", [128, 128], F32, kind="ExternalInput")
    w2 = nc.dram_tensor("w2", [128, 128], F32, kind="ExternalInput")
    bias = nc.dram_tensor("bias", [128, 3], F32, kind="ExternalInput")
    yT = nc.dram_tensor("yT", [128, MC], F32, kind="ExternalOutput")

    silu = mybir.ActivationFunctionType.Silu
    ident = mybir.ActivationFunctionType.Identity

    with tile.TileContext(nc) as tc:
        with (
            tc.tile_pool(name="const", bufs=1) as cp,
            tc.tile_pool(name="io", bufs=3) as io,
            tc.tile_pool(name="act", bufs=3) as ap,
            tc.tile_pool(name="psum", bufs=2, space="PSUM") as pp,
        ):
            wat = cp.tile([128, 128], F32)
            nc.sync.dma_start(wat[:], wa[:])
            wbt = cp.tile([128, 128], F32)
            nc.sync.dma_start(wbt[:], wb[:])
            wct = cp.tile([KMAX - 256, 128], F32)
            nc.sync.dma_start(wct[:], wc[:])
            w1t = cp.tile([128, 128], F32)
            nc.sync.dma_start(w1t[:], w1[:])
            w2t = cp.tile([128, 128], F32)
            nc.sync.dma_start(w2t[:], w2[:])
            bt = cp.tile([128, 3], F32)
            nc.sync.dma_start(bt[:], bias[:])

            for t in range(MC // TILE_N):
                sl = slice(t * TILE_N, (t + 1) * TILE_N)
                xa = io.tile([128, TILE_N], F32)
                nc.sync.dma_start(xa[:], xT[0:128, sl])
                xb = io.tile([128, TILE_N], F32)
                nc.sync.dma_start(xb[:], xT[128:256, sl])
                xc = io.tile([KMAX - 256, TILE_N], F32)
                nc.sync.dma_start(xc[:], xT[256:KMAX, sl])

                ps1 = pp.tile([128, TILE_N], F32, space="PSUM")
                nc.tensor.matmul(ps1[:], lhsT=wat[:], rhs=xa[:], start=True, stop=False)
                nc.tensor.matmul(ps1[:], lhsT=wbt[:], rhs=xb[:], start=False, stop=False)
                nc.tensor.matmul(ps1[:], lhsT=wct[:], rhs=xc[:], start=False, stop=True)
                h1 = ap.tile([128, TILE_N], F32)
                nc.scalar.activation(h1[:], ps1[:], silu, bias=bt[:, 0:1])

                ps2 = pp.tile([128, TILE_N], F32, space="PSUM")
                nc.tensor.matmul(ps2[:], lhsT=w1t[:], rhs=h1[:], start=True, stop=True)
                h2 = ap.tile([128, TILE_N], F32)
                nc.scalar.activation(h2[:], ps2[:], silu, bias=bt[:, 1:2])

                ps3 = pp.tile([128, TILE_N], F32, space="PSUM")
                nc.tensor.matmul(ps3[:], lhsT=w2t[:], rhs=h2[:], start=True, stop=True)
                h3 = ap.tile([128, TILE_N], F32)
                nc.scalar.activation(h3[:], ps3[:], ident, bias=bt[:, 2:3])

                nc.sync.dma_start(yT[:, sl], h3[:])
    nc.finalize()
    return nc


def _mlp_device(nc, x, mp):
    """y = LN(silu(silu(x@w0+b0)@w1+b1)@w2+b2) * g + be ; matmuls on device."""
    import time

    global LAST_HW_NS
    m, din = x.shape
    w0 = np.zeros((KMAX, 128), np.float32)
    w0[:din] = mp["w0"]
    shared = {
        "wa": np.ascontiguousarray(w0[0:128]),
        "wb": np.ascontiguousarray(w0[128:256]),
        "wc": np.ascontiguousarray(w0[256:KMAX]),
        "w1": np.ascontiguousarray(mp["w1"].astype(np.float32)),
        "w2": np.ascontiguousarray(mp["w2"].astype(np.float32)),
        "bias": np.ascontiguousarray(
            np.stack([mp["b0"], mp["b1"], mp["b2"]], axis=1).astype(np.float32)
        ),
    }
    per = -(-m // N_CORES)  # ceil
    assert per <= MC, (m, per)
    in_maps = []
    for c in range(N_CORES):
        xs = x[c * per : (c + 1) * per]
        xTc = np.zeros((KMAX, MC), np.float32)
        if xs.shape[0]:
            xTc[:din, : xs.shape[0]] = xs.T
        in_maps.append({**shared, "xT": xTc})
    t0 = time.time()
    res = run_bass_kernel_spmd(nc, in_maps, core_ids=list(range(N_CORES)))
    LAST_HW_NS += int((time.time() - t0) * 1e9)
    parts = []
    for c in range(N_CORES):
        take = max(0, min(per, m - c * per))
        parts.append(res.results[c]["yT"][:, :take].T)
    h = np.concatenate(parts, axis=0)
    mu = h.mean(-1, keepdims=True)
    var = ((h - mu) ** 2).mean(-1, keepdims=True)
    return (h - mu) / np.sqrt(var + 1e-5) * mp["g"] + mp["be"]


def _gmp(nc, p, x, g, pos):
    i, j = g[0], g[1]
    d = pos[i] - pos[j]
    nrm = np.linalg.norm(d, axis=-1, keepdims=True)
    tmp = np.concatenate([d, nrm, x[i], x[j]], axis=-1).astype(np.float32)
    e = _mlp_device(nc, tmp, p["edge"])
    aggr = np.zeros((x.shape[0], LAT), np.float32)
    np.add.at(aggr, j, e)
    nd = _mlp_device(nc, np.concatenate([x, aggr], axis=-1), p["node"])
    return nd + x


def _cal_ew(w, g, n):
    i, j = g[0], g[1]
    deg = np.bincount(i, minlength=n).astype(np.float32)
    normed = w / deg
    ws = normed[i]
    aggr = (np.bincount(j, weights=ws, minlength=n) + 1e-12).astype(np.float32)
    return (ws / aggr[j]).astype(np.float32), aggr


def _wconv(x, g, ew, aggregating, n):
    i, j = g[0], g[1]
    src, tgt = (i, j) if aggregating else (j, i)
    msg = x[src] * ew[:, None]
    out = np.zeros((n, x.shape[1]), np.float32)
    np.add.at(out, tgt, msg)
    return out


def _tree_np(o):
    if isinstance(o, dict):
        return {k: _tree_np(v) for k, v in o.items()}
    if isinstance(o, (list, tuple)):
        return [_tree_np(v) for v in o]
    return np.asarray(o)


def kernel(h, pos, g0, g1, g2, id0, id1, params):
    h = np.asarray(h, np.float32)
    pos = np.asarray(pos, np.float32)
    gs = [np.asarray(g0), np.asarray(g1), np.asarray(g2)]
    ids = [np.asarray(id0), np.asarray(id1)]
    p = _tree_np(params)
    nc = _build_kernel()

    w = np.ones((pos.shape[0],), np.float32)
    douts, dps, cts = [], [], []
    for l in range(DEPTH):
        n = h.shape[0]
        h = _gmp(nc, p["down"][l], h, gs[l], pos)
        douts.append(h)
        dps.append(pos)
        ew, w = _cal_ew(w, gs[l], n)
        h = _wconv(h, gs[l], ew, True, n)
        pos = _wconv(pos, gs[l], ew, True, n)
        cts.append(ew)
        h = h[ids[l]]
        pos = pos[ids[l]]
        w = w[ids[l]]
    h = _gmp(nc, p["bottom"], h, gs[DEPTH], pos)
    for l in range(DEPTH):
        d = DEPTH - 1 - l
        pre_n = douts[d].shape[0]
        hu = np.zeros((pre_n, h.shape[1]), np.float32)
        hu[ids[d]] = h
        h = _wconv(hu, gs[d], cts[d], False, pre_n)
        h = _gmp(nc, p["up"][l], h, gs[d], dps[d])
        h = h + douts[d]
    return h.astype(np.float32)
